# revision 9
# baseline (speedup 1.0000x reference)
"""Masked cross-attention (B=4, NQ=100, HW=4096, D=1024, H=16) on 8 TRN2 cores.

Sharding: kv rows (keys) are split 8 ways; each core runs LayerNorm + K/V
projection on its 512-key slice per batch, computes unnormalized partial
attention for all (b, h) against its keys, all-reduces the softmax
denominators (25.6 KB) on device, normalizes, and computes a partial
out-projection. The host sums the 8 partial outputs (the "all-reduce after
out_proj") and adds the folded bias.

LayerNorm gamma/beta are folded into the projection weights/biases on the
host; the V-projection bias is folded into the final output bias (exact
because softmax weights sum to one).
"""
import sys

sys.path.insert(0, "/opt/trn_rl_repo")

import numpy as np
import ml_dtypes

import concourse.bacc as bacc
import concourse.bass as bass
import concourse.mybir as mybir
import concourse.tile as tile
from concourse.bass_utils import run_bass_kernel_spmd
from concourse.masks import make_identity

B, NQ, HW, D, H = 4, 100, 4096, 1024, 16
HD = D // H          # 64
NCORE = 8
KC = HW // NCORE     # 512 keys per core per batch
NKT = KC // 128      # 4 key sub-tiles of 128
NDC = D // 128       # 8 chunks of the model dim
EPS = 1e-5
SCALE = 1.0 / np.sqrt(np.float32(HD))  # 1/8

F32 = mybir.dt.float32
BF16 = mybir.dt.bfloat16
AF = mybir.ActivationFunctionType
ALU = mybir.AluOpType

_compiled = {}


def _build():
    nc = bacc.Bacc("TRN2", target_bir_lowering=False, num_devices=NCORE)

    kv_d = nc.dram_tensor("kv", [B, NKT, 128, D], F32, kind="ExternalInput")
    q_d = nc.dram_tensor("q", [B, NQ, D], F32, kind="ExternalInput")
    mask_d = nc.dram_tensor("maskT", [B, 128, NKT, NQ], BF16, kind="ExternalInput")
    wq_d = nc.dram_tensor("wqT", [NDC, 128, D], BF16, kind="ExternalInput")
    wk_d = nc.dram_tensor("wkT", [NDC, 128, D], BF16, kind="ExternalInput")
    wv_d = nc.dram_tensor("wvT", [NDC, 128, D], BF16, kind="ExternalInput")
    wo_d = nc.dram_tensor("woT", [NDC, 128, D], BF16, kind="ExternalInput")
    bq_d = nc.dram_tensor("biasq", [NDC, 128], F32, kind="ExternalInput")
    bk_d = nc.dram_tensor("biask", [NDC, 128], F32, kind="ExternalInput")
    out_d = nc.dram_tensor("out", [B, 128, NDC, NQ], F32, kind="ExternalOutput")

    with tile.TileContext(nc) as tc:
        with (
            tc.tile_pool(name="sb", bufs=1) as sb,
            tc.tile_pool(name="ps", bufs=1, space="PSUM") as ps,
            tc.tile_pool(name="dram", bufs=1, space="DRAM") as dram,
        ):
            # ---- constants ----
            ident = sb.tile([128, 128], BF16, tag="ident")
            make_identity(nc, ident[:])
            eps_t = sb.tile([128, 1], F32, tag="eps")
            nc.vector.memset(eps_t[:], EPS)

            wk_sb, wv_sb, bq_sb, bk_sb = [], [], [], []
            for j in range(NDC):
                t = sb.tile([128, D], BF16, tag=f"wk{j}", name=f"wk{j}")
                nc.sync.dma_start(t[:], wk_d[j])
                wk_sb.append(t)
                t = sb.tile([128, D], BF16, tag=f"wv{j}", name=f"wv{j}")
                nc.sync.dma_start(t[:], wv_d[j])
                wv_sb.append(t)
                t = sb.tile([128, 1], F32, tag=f"bq{j}", name=f"bq{j}")
                nc.sync.dma_start(t[:], bq_d[j].unsqueeze(1))
                bq_sb.append(t)
                t = sb.tile([128, 1], F32, tag=f"bk{j}", name=f"bk{j}")
                nc.sync.dma_start(t[:], bk_d[j].unsqueeze(1))
                bk_sb.append(t)

            # wq tiles and (later) wo tiles share one 8-slot tag: wq is dead
            # after the Q projection, freeing the slots for wo.
            wq_sb = []
            for j in range(NDC):
                t = sb.tile([128, D], BF16, tag="wqo", bufs=NDC, name=f"wq{j}")
                nc.sync.dma_start(t[:], wq_d[j])
                wq_sb.append(t)

            def layernorm_to_bf16(x_f32, xn_bf16, p):
                """(x - mean) * rsqrt(var + eps), row-wise over the free dim."""
                stats = sb.tile([128, 2, 6], F32, tag="lnstats", bufs=4)
                nc.vector.bn_stats(stats[:p, 0, :], x_f32[:p, 0:512])
                nc.vector.bn_stats(stats[:p, 1, :], x_f32[:p, 512:1024])
                mv = sb.tile([128, 2], F32, tag="lnmv", bufs=4)
                nc.vector.bn_aggr(mv[:p], stats[:p])
                rstd = sb.tile([128, 1], F32, tag="lnrstd", bufs=4)
                nc.scalar.activation(rstd[:p], mv[:p, 1:2], AF.Sqrt, bias=eps_t[:p])
                nc.vector.reciprocal(rstd[:p], rstd[:p])
                nc.vector.tensor_scalar(
                    xn_bf16[:p], x_f32[:p], mv[:p, 0:1], rstd[:p],
                    ALU.subtract, ALU.mult,
                )

            # ---- stage 1: Q pipeline (all batches at once; rows = B*NQ) ----
            qnT = []
            for k in range(NDC):
                qnT.append(sb.tile([128, B, NQ], BF16, tag=f"qnT{k}", name=f"qnT{k}"))
            for b in range(B):
                qraw = sb.tile([NQ, D], F32, tag="qraw", bufs=2)
                nc.sync.dma_start(qraw[:], q_d[b])
                qn = sb.tile([NQ, D], BF16, tag="qn", bufs=2)
                layernorm_to_bf16(qraw, qn, NQ)
                for k in range(NDC):
                    tr = ps.tile([128, NQ], BF16, tag="tr", bufs=2)
                    nc.tensor.transpose(
                        tr[:], qn[:, k * 128:(k + 1) * 128], ident[:NQ, :NQ]
                    )
                    nc.vector.tensor_copy(out=qnT[k][:, b, :], in_=tr[:])

            qpT = []
            for j in range(NDC):
                qpT.append(sb.tile([128, B * NQ], BF16, tag=f"qpT{j}", name=f"qpT{j}"))
                acc = ps.tile([128, B * NQ], F32, tag="sc", bufs=2)
                for k in range(NDC):
                    nc.tensor.matmul(
                        acc[:],
                        lhsT=wq_sb[k][:, j * 128:(j + 1) * 128],
                        rhs=qnT[k][:].rearrange("p b q -> p (b q)"),
                        start=(k == 0), stop=(k == NDC - 1),
                    )
                nc.scalar.activation(qpT[j][:], acc[:], AF.Identity, bias=bq_sb[j][:])

            # wo loads (reuse wq slots once free)
            wo_sb = []
            for j in range(NDC):
                t = sb.tile([128, D], BF16, tag="wqo", bufs=NDC, name=f"wo{j}")
                nc.sync.dma_start(t[:], wo_d[j])
                wo_sb.append(t)

            # ---- stage 2+3: per-batch KV pipeline + partial attention ----
            sloc = dram.tile([B, NQ, H], F32)
            sglob = dram.tile([B, NQ, H], F32)
            ctx_sb = []
            for b in range(B):
                ctx_sb.append(
                    sb.tile([NQ, H, HD + 1], F32, tag=f"ctx{b}", name=f"ctx{b}")
                )

            for b in range(B):
                # LayerNorm + transpose -> kvnT[k] : [128 din, NKT*128 keys]
                kvnT = []
                for k in range(NDC):
                    kvnT.append(
                        sb.tile([128, NKT, 128], BF16, tag=f"kvnT{k}", bufs=2,
                                name=f"kvnT{k}_{b}")
                    )
                for r in range(NKT):
                    kvraw = sb.tile([128, D], F32, tag="kvraw", bufs=3)
                    nc.sync.dma_start(kvraw[:], kv_d[b, r])
                    xn = sb.tile([128, D], BF16, tag="xn", bufs=3)
                    layernorm_to_bf16(kvraw, xn, 128)
                    for k in range(NDC):
                        tr = ps.tile([128, 128], BF16, tag="tr", bufs=2)
                        nc.tensor.transpose(
                            tr[:], xn[:, k * 128:(k + 1) * 128], ident[:]
                        )
                        nc.vector.tensor_copy(out=kvnT[k][:, r, :], in_=tr[:])

                # K projection -> kpT[j] : [128 dout, KC keys]  (+bias via ACT)
                kpT = []
                for j in range(NDC):
                    kpT.append(
                        sb.tile([128, KC], BF16, tag=f"kpT{j}", bufs=2,
                                name=f"kpT{j}_{b}")
                    )
                    acc = ps.tile([128, KC], F32, tag="mm", bufs=2)
                    for k in range(NDC):
                        nc.tensor.matmul(
                            acc[:],
                            lhsT=wk_sb[k][:, j * 128:(j + 1) * 128],
                            rhs=kvnT[k][:].rearrange("p r k -> p (r k)"),
                            start=(k == 0), stop=(k == NDC - 1),
                        )
                    nc.scalar.activation(
                        kpT[j][:], acc[:], AF.Identity, bias=bk_sb[j][:]
                    )

                # V projection -> vp_ext[r] : [128 keys, H, HD+1], col HD = 1.0
                vp_ext = []
                for r in range(NKT):
                    vpe = sb.tile([128, H, HD + 1], BF16, tag=f"vpe{r}", bufs=2,
                                  name=f"vpe{r}_{b}")
                    vp_ext.append(vpe)
                    nc.vector.memset(vpe[:, :, HD:HD + 1], 1.0)
                    for nh in range(2):
                        acc = ps.tile([128, 512], F32, tag="mm", bufs=2)
                        for k in range(NDC):
                            nc.tensor.matmul(
                                acc[:],
                                lhsT=kvnT[k][:, r, :],
                                rhs=wv_sb[k][:, nh * 512:(nh + 1) * 512],
                                start=(k == 0), stop=(k == NDC - 1),
                            )
                        nc.vector.tensor_copy(
                            out=vpe[:, nh * 8:(nh + 1) * 8, 0:HD],
                            in_=acc[:].rearrange("p (g d) -> p g d", g=8),
                        )

                # mask slice for this batch: [128, NKT, NQ] bf16 (0/1)
                mask_b = sb.tile([128, NKT, NQ], BF16, tag="maskb", bufs=2)
                nc.sync.dma_start(mask_b[:], mask_d[b])

                # scores^T, exp, mask, ctx per head
                exp_all = sb.tile([128, NKT, H, NQ], BF16, tag="expall", bufs=2)
                for h in range(H):
                    j, off = h // 2, (h % 2) * HD
                    sc = ps.tile([128, NKT, NQ], F32, tag="sc", bufs=2)
                    for c in range(NKT):
                        nc.tensor.matmul(
                            sc[:, c, :],
                            lhsT=kpT[j][off:off + HD, c * 128:(c + 1) * 128],
                            rhs=qpT[j][off:off + HD, b * NQ:(b + 1) * NQ],
                            start=True, stop=True,
                        )
                    nc.scalar.activation(
                        exp_all[:, :, h, :], sc[:], AF.Exp, scale=float(SCALE)
                    )
                    nc.vector.tensor_mul(
                        exp_all[:, :, h, :], exp_all[:, :, h, :], mask_b[:]
                    )
                    ctx_ps = ps.tile([NQ, HD + 1], F32, tag="ctx", bufs=2)
                    for c in range(NKT):
                        nc.tensor.matmul(
                            ctx_ps[:],
                            lhsT=exp_all[:, c, h, :],
                            rhs=vp_ext[c][:, h, :],
                            start=(c == 0), stop=(c == NKT - 1),
                        )
                    nc.vector.tensor_copy(out=ctx_sb[b][:, h, :], in_=ctx_ps[:])

                # local softmax denominators -> DRAM for the all-reduce
                nc.sync.dma_start(sloc[b], ctx_sb[b][:, :, HD])

            # ---- stage 4: all-reduce denominators, normalize, out-proj ----
            nc.gpsimd.collective_compute(
                "AllReduce", ALU.add,
                replica_groups=[list(range(NCORE))],
                ins=[sloc.opt()], outs=[sglob.opt()],
            )
            sg_sb = sb.tile([NQ, B, H], F32, tag="sglob")
            nc.sync.dma_start(sg_sb[:], sglob[:].transpose([1, 0, 2]))
            recip = sb.tile([NQ, B, H], F32, tag="recip")
            nc.vector.reciprocal(recip[:], sg_sb[:])

            for b in range(B):
                ctxn = sb.tile([NQ, H, HD], BF16, tag="ctxn", bufs=2)
                for h in range(H):
                    nc.vector.tensor_scalar_mul(
                        ctxn[:, h, :], ctx_sb[b][:, h, 0:HD], recip[:, b, h:h + 1]
                    )
                ctxT = []
                for j in range(NDC):
                    t = sb.tile([128, NQ], BF16, tag=f"ctxT{j}", bufs=2,
                                name=f"ctxT{j}_{b}")
                    ctxT.append(t)
                    tr = ps.tile([128, NQ], BF16, tag="tr", bufs=2)
                    for hh in range(2):
                        nc.tensor.transpose(
                            tr[hh * HD:(hh + 1) * HD, :],
                            ctxn[:, 2 * j + hh, :],
                            ident[:NQ, :NQ],
                        )
                    nc.vector.tensor_copy(out=t[:], in_=tr[:])

                out_sb = sb.tile([128, NDC, NQ], F32, tag="outsb", bufs=2)
                for m in range(NDC):
                    acc = ps.tile([128, NQ], F32, tag="sc", bufs=2)
                    for k in range(NDC):
                        nc.tensor.matmul(
                            acc[:],
                            lhsT=wo_sb[k][:, m * 128:(m + 1) * 128],
                            rhs=ctxT[k][:],
                            start=(k == 0), stop=(k == NDC - 1),
                        )
                    nc.any.tensor_copy(out=out_sb[:, m, :], in_=acc[:])
                nc.sync.dma_start(out_d[b], out_sb[:])

    nc.compile()
    return nc


def _prep_in_maps(q, kv, mask, in_proj_w, in_proj_b, out_w, out_b,
                  g_q, b_q, g_kv, b_kv):
    """Host-side prep: fold LN affine + V-bias, shard kv/mask per core.

    Returns (in_maps, bias_total)."""
    q = np.asarray(q, np.float32)
    kv = np.asarray(kv, np.float32)
    mask = np.asarray(mask)
    in_proj_w = np.asarray(in_proj_w, np.float32)
    in_proj_b = np.asarray(in_proj_b, np.float32)
    out_w = np.asarray(out_w, np.float32)
    out_b = np.asarray(out_b, np.float32)
    g_q = np.asarray(g_q, np.float32)
    b_q = np.asarray(b_q, np.float32)
    g_kv = np.asarray(g_kv, np.float32)
    b_kv = np.asarray(b_kv, np.float32)

    Wq, Wk, Wv = in_proj_w[:D], in_proj_w[D:2 * D], in_proj_w[2 * D:]
    bq, bk, bv = in_proj_b[:D], in_proj_b[D:2 * D], in_proj_b[2 * D:]

    # Fold LayerNorm affine into projections: LN(x)*g+b @ W^T + c
    #   = LN(x) @ (W*g)^T + (W@b + c)
    WqT = (Wq * g_q[None, :]).T.astype(ml_dtypes.bfloat16)
    WkT = (Wk * g_kv[None, :]).T.astype(ml_dtypes.bfloat16)
    WvT = (Wv * g_kv[None, :]).T.astype(ml_dtypes.bfloat16)
    bq_eff = (bq + Wq @ b_q).astype(np.float32)
    bk_eff = (bk + Wk @ b_kv).astype(np.float32)
    bv_eff = (bv + Wv @ b_kv).astype(np.float32)
    # V bias passes through softmax unchanged (weights sum to 1): fold into
    # the final output bias on the host.
    WoT = out_w.T.astype(ml_dtypes.bfloat16)
    bias_total = (out_b + out_w @ bv_eff).astype(np.float32)

    # per-query key mask; all-zero mask rows attend everywhere
    allowed = (mask != 0)
    has_any = allowed.any(axis=-1, keepdims=True)
    eff = np.where(has_any, allowed, True)  # [B, NQ, HW] bool

    common = {
        "q": np.ascontiguousarray(q),
        "wqT": np.ascontiguousarray(WqT.reshape(NDC, 128, D)),
        "wkT": np.ascontiguousarray(WkT.reshape(NDC, 128, D)),
        "wvT": np.ascontiguousarray(WvT.reshape(NDC, 128, D)),
        "woT": np.ascontiguousarray(WoT.reshape(NDC, 128, D)),
        "biasq": np.ascontiguousarray(bq_eff.reshape(NDC, 128)),
        "biask": np.ascontiguousarray(bk_eff.reshape(NDC, 128)),
    }
    in_maps = []
    for c in range(NCORE):
        sl = slice(c * KC, (c + 1) * KC)
        kv_c = kv[:, sl, :].reshape(B, NKT, 128, D)
        # mask slice -> [B, 128, NKT, NQ] bf16 (keysub-tile on partitions)
        m_c = eff[:, :, sl].transpose(0, 2, 1).reshape(B, NKT, 128, NQ)
        m_c = m_c.transpose(0, 2, 1, 3).astype(ml_dtypes.bfloat16)
        in_maps.append({
            **common,
            "kv": np.ascontiguousarray(kv_c),
            "maskT": np.ascontiguousarray(m_c),
        })
    return in_maps, bias_total


def kernel(q, kv, mask, in_proj_w, in_proj_b, out_w, out_b, g_q, b_q, g_kv, b_kv):
    in_maps, bias_total = _prep_in_maps(
        q, kv, mask, in_proj_w, in_proj_b, out_w, out_b, g_q, b_q, g_kv, b_kv
    )
    if "nc" not in _compiled:
        _compiled["nc"] = _build()
    nc = _compiled["nc"]

    res = run_bass_kernel_spmd(nc, in_maps, core_ids=list(range(NCORE)))

    out = np.zeros((B, NQ, D), np.float32)
    for c in range(NCORE):
        part = res.results[c]["out"]  # [B, 128, NDC, NQ]
        out += part.transpose(0, 3, 2, 1).reshape(B, NQ, D)
    out += bias_total[None, None, :]
    return out
